# revision 18
# baseline (speedup 1.0000x reference)
"""Trainium2 Bass kernel for nn_ActionTensorLoss.

Reference semantics (B=4096 samples, A=2048 max actions, F=8 features):
  predictions/targets: (B, A+1, F) f32; [:,0,0] carries the action count.
  count_loss  = mean((pred_counts - target_counts)^2)
  per-sample  = sum((pred_acts - targ_acts)^2 * row_mask) / max(8*count, 1)
  total = count_loss + 2 * sum(per_sample) / max(n_valid, 1)   [if n_valid>0]

Sharding: pure data-parallel over the batch dim across 8 NeuronCores
(512 samples/core). Each core reduces its shard to per-sample-lane partials
out[128, 3] = (sum count_sq, sum per_sample_mse, sum valid) accumulated over
4 groups of 128 samples; the host sums 8*128 lanes and applies the final
scalar arithmetic.

Per-core pipeline (memory-bound; ~67 MB of HBM reads per core):
  for each group (128 samples on partitions) and each of 3 column chunks
  (683 rows * 8 feats = 5464 f32 per partition):
    DMA  targ chunk, pred chunk                       (sync/HWDGE)
    DVE  diff = pred - targ            (tensor_tensor, in place)
    DVE  md = (iota < 8*(count+1)) * diff  (scalar_tensor_tensor)
    ACT  Square(md), accum_out -> per-chunk masked sum of squares
  The iota constant for chunk 0 has positions 0..7 poisoned (+1e9) so the
  header row is excluded from the action loss automatically.
"""

import os
from contextlib import ExitStack

import numpy as np

B, A, F = 4096, 2048, 8
ROWS = A + 1            # 2049 rows (header + actions)
FREE = ROWS * F         # 16392 f32 per sample
N_CORES = 8
B_CORE = B // N_CORES   # 512 samples per core
P = 128                 # partitions
GROUPS = B_CORE // P    # 4
_chunks_env = os.environ.get("ATL_CHUNKS", "")
if _chunks_env:
    _base = [int(x) for x in _chunks_env.split(",")]
    CHUNKS_BY_GROUP = [_base] * GROUPS
else:
    # Big chunks amortize per-op and per-DMA overheads; only the very last
    # chunk of the last group is small so the post-final-DMA compute tail
    # (diff + mask + square on that chunk) is short.
    CHUNKS_BY_GROUP = [[5464, 5464, 5464]] * (GROUPS - 1) + [
        [5464, 5464, 4781, 683]
    ]
for _c in CHUNKS_BY_GROUP:
    assert sum(_c) == FREE, (sum(_c), FREE)
MAXCH = max(max(c) for c in CHUNKS_BY_GROUP)
IO_BUFS = int(os.environ.get("ATL_IO_BUFS", "3"))
DMA_SPLIT = os.environ.get("ATL_DMA_SPLIT", "1") == "1"  # pred DMAs on ACT ring
W_ACTION_COUNT = 1.0
W_ACTION_TENSOR = 2.0

_CACHED_NC = None


def _build():
    import concourse.bass as bass  # noqa: F401
    import concourse.tile as tile
    from concourse import bacc, mybir

    f32 = mybir.dt.float32
    Alu = mybir.AluOpType
    ActF = mybir.ActivationFunctionType

    nc = bacc.Bacc(
        "TRN2",
        target_bir_lowering=False,
        debug=False,
        num_devices=N_CORES,
        enable_partition_id=os.environ.get("ATL_NO_PID", "0") != "1",
    )
    pred = nc.dram_tensor(
        "predictions", [B_CORE, FREE], f32, kind="ExternalInput"
    ).ap()
    targ = nc.dram_tensor("targets", [B_CORE, FREE], f32, kind="ExternalInput").ap()
    out = nc.dram_tensor("out", [P, 3], f32, kind="ExternalOutput").ap()

    with ExitStack() as ctx:
        tc = ctx.enter_context(tile.TileContext(nc))
        io = ctx.enter_context(tc.tile_pool(name="io", bufs=IO_BUFS))
        consts = ctx.enter_context(tc.tile_pool(name="consts", bufs=1))
        small = ctx.enter_context(tc.tile_pool(name="small", bufs=2))
        accp = ctx.enter_context(tc.tile_pool(name="acc", bufs=1))

        # iota constants: values 0..MAXCH-1 (exact in f32), plus a copy whose
        # first 8 entries (the header row in chunk 0) are poisoned so the
        # is_lt mask always drops them.
        iota_f = consts.tile([P, MAXCH], f32, tag="iota_f")
        nc.gpsimd.iota(
            iota_f[:], [[1, MAXCH]], channel_multiplier=0,
            allow_small_or_imprecise_dtypes=True,
        )
        iota_p = consts.tile([P, MAXCH], f32, tag="iota_p")
        nc.vector.tensor_copy(iota_p[:], iota_f[:])
        nc.vector.memset(iota_p[:, 0:8], 1.0e9)

        vg = [accp.tile([P, 3], f32, tag=f"vg{g}", name=f"vg{g}") for g in range(GROUPS)]
        acc = [
            [accp.tile([P, 1], f32, tag=f"ac{g}_{c}", name=f"ac{g}_{c}")
             for c in range(len(CHUNKS_BY_GROUP[g]))]
            for g in range(GROUPS)
        ]

        for g in range(GROUPS):
            CHUNKS = CHUNKS_BY_GROUP[g]
            NCH = len(CHUNKS)
            r0 = g * P
            c_g = small.tile([P, 1], f32, tag="c")
            e_loc = [small.tile([P, 1], f32, tag=f"e{i}", name=f"e{g}_{i}") for i in range(NCH)]
            off = 0
            for chx, L in enumerate(CHUNKS):
                t_t = io.tile([P, MAXCH], f32, tag="t")
                p_t = io.tile([P, MAXCH], f32, tag="p")
                tt = t_t[:, 0:L]
                pt = p_t[:, 0:L]
                nc.sync.dma_start(tt, targ[r0 : r0 + P, off : off + L])
                p_dma = nc.scalar if DMA_SPLIT else nc.sync
                p_dma.dma_start(pt, pred[r0 : r0 + P, off : off + L])
                if chx == 0:
                    # count lives at [row 0, feat 0] of the targets chunk
                    nc.vector.tensor_copy(c_g[:], t_t[:, 0:1])
                    # E0 = 8*(c+1); E_i = E0 - off_i (mask threshold, local)
                    nc.vector.tensor_scalar(
                        e_loc[0][:], c_g[:], 8.0, 8.0, Alu.mult, Alu.add
                    )
                    o = 0
                    for i in range(1, NCH):
                        o += CHUNKS[i - 1]
                        nc.vector.tensor_scalar(
                            e_loc[i][:], e_loc[0][:], float(o), None,
                            Alu.subtract,
                        )
                # diff = pred - targ, in place over the targets tile
                nc.vector.tensor_tensor(tt, pt, tt, Alu.subtract)
                if chx == 0:
                    # count-loss contribution: diff[0]^2
                    nc.vector.tensor_tensor(
                        vg[g][:, 0:1], t_t[:, 0:1], t_t[:, 0:1], Alu.mult
                    )
                iota_t = iota_p if chx == 0 else iota_f
                # masked diff: (iota < E) * diff, in place
                nc.vector.scalar_tensor_tensor(
                    tt, iota_t[:, 0:L], e_loc[chx][:], tt, Alu.is_lt, Alu.mult
                )
                # sum of squares of the masked diff -> acc[g][chx]
                nc.scalar.activation(
                    pt, tt, ActF.Square, accum_out=acc[g][chx][:]
                )
                off += L
            # group epilogue: per-sample mse and validity
            asum = small.tile([P, 1], f32, tag="asum")
            nc.vector.tensor_tensor(asum[:], acc[g][0][:], acc[g][1][:], Alu.add)
            for i in range(2, NCH):
                nc.vector.tensor_tensor(asum[:], asum[:], acc[g][i][:], Alu.add)
            den = small.tile([P, 1], f32, tag="den")
            nc.vector.tensor_scalar(den[:], c_g[:], 8.0, 1.0, Alu.mult, Alu.max)
            rcp = small.tile([P, 1], f32, tag="rcp")
            nc.vector.reciprocal(rcp[:], den[:])
            nc.vector.tensor_tensor(vg[g][:, 1:2], asum[:], rcp[:], Alu.mult)
            nc.vector.tensor_scalar(
                vg[g][:, 2:3], c_g[:], 0.5, None, Alu.is_ge
            )
        # combine the 4 groups -> out[128, 3]
        v01 = accp.tile([P, 3], f32, tag="v01")
        v23 = accp.tile([P, 3], f32, tag="v23")
        nc.vector.tensor_tensor(v01[:], vg[0][:], vg[1][:], Alu.add)
        nc.vector.tensor_tensor(v23[:], vg[2][:], vg[3][:], Alu.add)
        nc.vector.tensor_tensor(v01[:], v01[:], v23[:], Alu.add)
        nc.sync.dma_start(out[:], v01[:])

    nc.compile()
    return nc


def get_nc():
    global _CACHED_NC
    if _CACHED_NC is None:
        _CACHED_NC = _build()
    return _CACHED_NC


def _make_in_maps(predictions, targets):
    p = np.asarray(predictions, dtype=np.float32).reshape(B, FREE)
    t = np.asarray(targets, dtype=np.float32).reshape(B, FREE)
    in_maps = []
    for i in range(N_CORES):
        sl = slice(i * B_CORE, (i + 1) * B_CORE)
        in_maps.append(
            {
                "predictions": np.ascontiguousarray(p[sl]),
                "targets": np.ascontiguousarray(t[sl]),
            }
        )
    return in_maps


def _combine(core_outs):
    vals = np.stack(core_outs).astype(np.float64)  # (8, 128, 3)
    csq_sum = vals[..., 0].sum()
    mse_sum = vals[..., 1].sum()
    n_valid = vals[..., 2].sum()
    count_loss = csq_sum / B
    atl = mse_sum / max(n_valid, 1.0)
    total = W_ACTION_COUNT * count_loss + (
        W_ACTION_TENSOR * atl if n_valid > 0 else 0.0
    )
    return np.array(total, dtype=np.float32)


def _ensure_ntff_hook():
    """The agent image's antenv package lacks axon_hooks, so the boot-time
    NTFF hook registration silently degrades. Recreate the module shim and
    register the ctypes hook so trace=True produces exec_time_ns."""
    import sys
    import types

    try:
        from antenv.axon_hooks import get_axon_ntff_profile_hook  # noqa: F401
        return
    except ImportError:
        pass
    mod = types.ModuleType("antenv.axon_hooks")
    _hook = [None]
    mod.set_axon_ntff_profile_hook = lambda h: _hook.__setitem__(0, h)
    mod.get_axon_ntff_profile_hook = lambda: _hook[0]
    sys.modules["antenv.axon_hooks"] = mod
    import antenv

    antenv.axon_hooks = mod
    try:
        from trn_agent_boot.trn_boot import _ntff_profile_via_ctypes

        mod.set_axon_ntff_profile_hook(
            _ntff_profile_via_ctypes("/opt/axon/libaxon_pjrt.so")
        )
    except Exception:
        pass


_STAGED_CACHE = {}


def _staged_exec(nc, in_maps, trace=False):
    """Execute the compiled Bass module via PJRT with inputs pre-staged onto
    the devices (device_put + block) before the NEFF launch.  The stock
    run_bass_via_pjrt path feeds numpy arrays straight into jit, so the
    537 MB of host->device traffic overlaps the NEFF execution; core 0
    (which receives its shard first and shares an HBM stack with core 1)
    then executes under HBM contention and runs ~20% slower.  Staging first
    keeps the measured kernel window clean."""
    import jax
    import numpy as np
    from jax.sharding import Mesh, NamedSharding, PartitionSpec
    from jax.experimental.shard_map import shard_map
    from concourse import bass2jax, mybir
    from concourse.bass2jax import _bass_exec_p, install_neuronx_cc_hook

    install_neuronx_cc_hook()
    n_cores = len(in_maps)

    partition_name = (
        nc.partition_id_tensor.name if nc.partition_id_tensor else None
    )
    in_names, out_names, out_avals, zero_outs = [], [], [], []
    for alloc in nc.m.functions[0].allocations:
        if not isinstance(alloc, mybir.MemoryLocationSet):
            continue
        name = alloc.memorylocations[0].name
        if alloc.kind == "ExternalInput":
            if name != partition_name:
                in_names.append(name)
        elif alloc.kind == "ExternalOutput":
            shape = tuple(alloc.tensor_shape)
            dtype = mybir.dt.np(alloc.dtype)
            out_names.append(name)
            out_avals.append(jax.core.ShapedArray(shape, dtype))
            zero_outs.append(np.zeros(shape, dtype))
    n_params = len(in_names)
    all_in_names = in_names + out_names
    if partition_name is not None:
        all_in_names.append(partition_name)
    donate = tuple(range(n_params, n_params + len(out_names)))

    def _body(*args):
        operands = list(args)
        if partition_name is not None:
            operands.append(bass2jax.partition_id_tensor())
        outs = _bass_exec_p.bind(
            *operands,
            out_avals=tuple(out_avals),
            in_names=tuple(all_in_names),
            out_names=tuple(out_names),
            lowering_input_output_aliases=(),
            sim_require_finite=True,
            sim_require_nnan=True,
            nc=nc,
        )
        return tuple(outs)

    devices = jax.devices()[:n_cores]
    mesh = Mesh(np.asarray(devices), ("core",))
    sharding = NamedSharding(mesh, PartitionSpec("core"))
    if id(nc) not in _STAGED_CACHE:
        sharded = jax.jit(
            shard_map(
                _body,
                mesh=mesh,
                in_specs=(PartitionSpec("core"),) * (n_params + len(out_names)),
                out_specs=(PartitionSpec("core"),) * len(out_names),
                check_rep=False,
            ),
            donate_argnums=donate,
            keep_unused=True,
        )
        _STAGED_CACHE[id(nc)] = sharded
    sharded = _STAGED_CACHE[id(nc)]

    concat_in = [
        np.concatenate([np.asarray(m[name]) for m in in_maps], axis=0)
        for name in in_names
    ]
    staged = [jax.device_put(a, sharding) for a in concat_in]
    staged_zeros = [
        jax.device_put(
            np.zeros((n_cores * z.shape[0], *z.shape[1:]), z.dtype), sharding
        )
        for z in zero_outs
    ]
    jax.block_until_ready(staged)
    jax.block_until_ready(staged_zeros)

    out_arrs = sharded(*staged, *staged_zeros)
    jax.block_until_ready(out_arrs)
    return [
        {
            name: np.asarray(out_arrs[i]).reshape(n_cores, *out_avals[i].shape)[c]
            for i, name in enumerate(out_names)
        }
        for c in range(n_cores)
    ]


def _run_staged(nc, in_maps, trace=False, trace_cores=None, warmup=False):
    """Staged execution; with trace=True, captures NTFF on a warm, quiet run."""
    import glob
    import tempfile

    from concourse import bass_utils

    if not trace:
        results = _staged_exec(nc, in_maps)
        return bass_utils.BassKernelResults(
            results=results,
            instructions_and_trace=None,
            profile_json=None,
            exec_time_ns=None,
        )

    _ensure_ntff_hook()
    from antenv.axon_hooks import get_axon_ntff_profile_hook

    hook = get_axon_ntff_profile_hook()
    if hook is None:
        raise RuntimeError("no ntff hook")
    if warmup:
        _staged_exec(nc, in_maps)  # compile + warm the devices
    neff_dir = tempfile.mkdtemp()
    idxs = list(trace_cores) if trace_cores is not None else list(range(N_CORES))
    with hook(neff_dir, idxs):
        results = _staged_exec(nc, in_maps)
    ntffs = glob.glob(os.path.join(neff_dir, "*_body*.ntff"))
    if not ntffs:
        return bass_utils.BassKernelResults(
            results=results,
            instructions_and_trace=None,
            profile_json=None,
            exec_time_ns=None,
        )
    import gauge.profiler
    from concourse._compat import FishPath

    sharepath = bass_utils.upload_artifacts(neff_dir)
    profile = gauge.profiler.Profile(
        profile_path=FishPath(neff_dir),
        kernel_dev_mode=True,
        profile_on_exit=False,
        bass_kernel=nc.m,
        offline_processing=True,
        fname="*_body*",
        metadata={"artifacts_path": sharepath},
    )
    perf = bass_utils._process_ntff_profile(
        profile,
        neff_dir,
        nc,
        list(range(N_CORES)),
        trace_cores,
        False,
        {},
        trace_events=False,
    )
    return perf.as_bass_kernel_results(results)


def run(predictions, targets, trace=False, staged=None, trace_cores=None,
        warmup=False, **spmd_kwargs):
    """Run on the 8 NeuronCores; returns (output, BassKernelResults)."""
    from concourse.bass_utils import run_bass_kernel_spmd

    nc = get_nc()
    in_maps = _make_in_maps(predictions, targets)

    trace = trace or os.environ.get("BASS_TRACE", "") == "1"
    if staged is None:
        staged = os.environ.get("ATL_STAGED", "1") == "1"
    if staged:
        try:
            res = _run_staged(
                nc, in_maps, trace=trace, trace_cores=trace_cores, warmup=warmup
            )
            out = _combine([r["out"] for r in res.results])
            return out, res
        except Exception:
            import traceback

            traceback.print_exc()

    if trace:
        _ensure_ntff_hook()
    res = run_bass_kernel_spmd(
        nc, in_maps, core_ids=list(range(N_CORES)), trace=trace, **spmd_kwargs
    )
    out = _combine([r["out"] for r in res.results])
    return out, res


def kernel(predictions, targets):
    out, _ = run(predictions, targets, trace=False)
    return out


if __name__ == "__main__":
    np.random.seed(0)
    preds = np.random.randn(B, ROWS, F).astype(np.float32)
    targs = np.random.randn(B, ROWS, F).astype(np.float32)
    counts = np.random.randint(0, A + 1, size=B).astype(np.float32)
    targs[:, 0, 0] = counts
    print(kernel(preds, targs))


# revision 21
# speedup vs baseline: 1.0533x; 1.0533x over previous
"""Trainium2 Bass kernel for nn_ActionTensorLoss.

Reference semantics (B=4096 samples, A=2048 max actions, F=8 features):
  predictions/targets: (B, A+1, F) f32; [:,0,0] carries the action count.
  count_loss  = mean((pred_counts - target_counts)^2)
  per-sample  = sum((pred_acts - targ_acts)^2 * row_mask) / max(8*count, 1)
  total = count_loss + 2 * sum(per_sample) / max(n_valid, 1)   [if n_valid>0]

Sharding: pure data-parallel over the batch dim across 8 NeuronCores
(512 samples/core). Each core reduces its shard to per-sample-lane partials
out[128, 3] = (sum count_sq, sum per_sample_mse, sum valid) accumulated over
4 groups of 128 samples; the host sums 8*128 lanes and applies the final
scalar arithmetic.

Per-core pipeline (memory-bound; ~67 MB of HBM reads per core):
  for each group (128 samples on partitions) and each of 3 column chunks
  (683 rows * 8 feats = 5464 f32 per partition):
    DMA  targ chunk, pred chunk                       (sync/HWDGE)
    DVE  diff = pred - targ            (tensor_tensor, in place)
    DVE  md = (iota < 8*(count+1)) * diff  (scalar_tensor_tensor)
    ACT  Square(md), accum_out -> per-chunk masked sum of squares
  The iota constant for chunk 0 has positions 0..7 poisoned (+1e9) so the
  header row is excluded from the action loss automatically.
"""

import os
from contextlib import ExitStack

import numpy as np

B, A, F = 4096, 2048, 8
ROWS = A + 1            # 2049 rows (header + actions)
FREE = ROWS * F         # 16392 f32 per sample
N_CORES = 8
B_CORE = B // N_CORES   # 512 samples per core
P = 128                 # partitions
GROUPS = B_CORE // P    # 4
_chunks_env = os.environ.get("ATL_CHUNKS", "")
if _chunks_env:
    _base = [int(x) for x in _chunks_env.split(",")]
    CHUNKS_BY_GROUP = [_base] * GROUPS
else:
    # Big chunks amortize per-op and per-DMA overheads; the last group's
    # chunks taper off so the pipeline-drain tail after the final DMA is
    # short (the backlog at stream end is made of small chunks).
    CHUNKS_BY_GROUP = [[4098, 4098, 4098, 4098]] * (GROUPS - 1) + [
        [4098, 4098, 4098, 2049, 1366, 683]
    ]
for _c in CHUNKS_BY_GROUP:
    assert sum(_c) == FREE, (sum(_c), FREE)
MAXCH = max(max(c) for c in CHUNKS_BY_GROUP)
IO_BUFS = int(os.environ.get("ATL_IO_BUFS", "3"))
DMA_SPLIT = os.environ.get("ATL_DMA_SPLIT", "1") == "1"  # pred DMAs on ACT ring
W_ACTION_COUNT = 1.0
W_ACTION_TENSOR = 2.0

_CACHED_NC = None


def _build():
    import concourse.bass as bass  # noqa: F401
    import concourse.tile as tile
    from concourse import bacc, mybir

    f32 = mybir.dt.float32
    Alu = mybir.AluOpType
    ActF = mybir.ActivationFunctionType

    nc = bacc.Bacc(
        "TRN2",
        target_bir_lowering=False,
        debug=False,
        num_devices=N_CORES,
        enable_partition_id=os.environ.get("ATL_NO_PID", "0") != "1",
    )
    pred = nc.dram_tensor(
        "predictions", [B_CORE, FREE], f32, kind="ExternalInput"
    ).ap()
    targ = nc.dram_tensor("targets", [B_CORE, FREE], f32, kind="ExternalInput").ap()
    out = nc.dram_tensor("out", [P, 3], f32, kind="ExternalOutput").ap()

    with ExitStack() as ctx:
        tc = ctx.enter_context(tile.TileContext(nc))
        io = ctx.enter_context(tc.tile_pool(name="io", bufs=IO_BUFS))
        scrp = ctx.enter_context(tc.tile_pool(name="scr", bufs=2))
        consts = ctx.enter_context(tc.tile_pool(name="consts", bufs=1))
        small = ctx.enter_context(tc.tile_pool(name="small", bufs=2))
        accp = ctx.enter_context(tc.tile_pool(name="acc", bufs=1))

        # iota constants: values 0..MAXCH-1 (exact in f32), plus a copy whose
        # first 8 entries (the header row in chunk 0) are poisoned so the
        # is_lt mask always drops them.
        iota_f = consts.tile([P, MAXCH], f32, tag="iota_f")
        nc.gpsimd.iota(
            iota_f[:], [[1, MAXCH]], channel_multiplier=0,
            allow_small_or_imprecise_dtypes=True,
        )
        iota_p = consts.tile([P, MAXCH], f32, tag="iota_p")
        nc.vector.tensor_copy(iota_p[:], iota_f[:])
        nc.vector.memset(iota_p[:, 0:8], 1.0e9)

        vg = [accp.tile([P, 3], f32, tag=f"vg{g}", name=f"vg{g}") for g in range(GROUPS)]
        acc = [
            [accp.tile([P, 1], f32, tag=f"ac{g}_{c}", name=f"ac{g}_{c}")
             for c in range(len(CHUNKS_BY_GROUP[g]))]
            for g in range(GROUPS)
        ]

        for g in range(GROUPS):
            CHUNKS = CHUNKS_BY_GROUP[g]
            NCH = len(CHUNKS)
            r0 = g * P
            c_g = small.tile([P, 1], f32, tag="c")
            e_loc = [small.tile([P, 1], f32, tag=f"e{i}", name=f"e{g}_{i}") for i in range(NCH)]
            off = 0
            for chx, L in enumerate(CHUNKS):
                t_t = io.tile([P, MAXCH], f32, tag="t")
                p_t = io.tile([P, MAXCH], f32, tag="p")
                scr = scrp.tile([P, MAXCH], f32, tag="scr")
                tt = t_t[:, 0:L]
                pt = p_t[:, 0:L]
                sc = scr[:, 0:L]
                nc.sync.dma_start(tt, targ[r0 : r0 + P, off : off + L])
                p_dma = nc.scalar if DMA_SPLIT else nc.sync
                p_dma.dma_start(pt, pred[r0 : r0 + P, off : off + L])
                if chx == 0:
                    # count lives at [row 0, feat 0] of the targets chunk
                    nc.vector.tensor_copy(c_g[:], t_t[:, 0:1])
                    # E0 = 8*(c+1); E_i = E0 - off_i (mask threshold, local)
                    nc.vector.tensor_scalar(
                        e_loc[0][:], c_g[:], 8.0, 8.0, Alu.mult, Alu.add
                    )
                    o = 0
                    for i in range(1, NCH):
                        o += CHUNKS[i - 1]
                        nc.vector.tensor_scalar(
                            e_loc[i][:], e_loc[0][:], float(o), None,
                            Alu.subtract,
                        )
                # diff = pred - targ, in place over the predictions tile;
                # the targets tile is read-only here so its buffer recycles
                # as soon as this op finishes (DVE, not ACT, gates the DMAs)
                nc.vector.tensor_tensor(pt, pt, tt, Alu.subtract)
                if chx == 0:
                    # count-loss contribution: diff[0]^2
                    nc.vector.tensor_tensor(
                        vg[g][:, 0:1], p_t[:, 0:1], p_t[:, 0:1], Alu.mult
                    )
                iota_t = iota_p if chx == 0 else iota_f
                # masked diff into the scratch tile; frees the pred tile
                nc.vector.scalar_tensor_tensor(
                    sc, iota_t[:, 0:L], e_loc[chx][:], pt, Alu.is_lt, Alu.mult
                )
                # sum of squares of the masked diff -> acc[g][chx]
                # (in place on scratch; io tiles are not touched by ACT)
                nc.scalar.activation(
                    sc, sc, ActF.Square, accum_out=acc[g][chx][:]
                )
                off += L
            # group epilogue: per-sample mse and validity
            asum = small.tile([P, 1], f32, tag="asum")
            nc.vector.tensor_tensor(asum[:], acc[g][0][:], acc[g][1][:], Alu.add)
            for i in range(2, NCH):
                nc.vector.tensor_tensor(asum[:], asum[:], acc[g][i][:], Alu.add)
            den = small.tile([P, 1], f32, tag="den")
            nc.vector.tensor_scalar(den[:], c_g[:], 8.0, 1.0, Alu.mult, Alu.max)
            rcp = small.tile([P, 1], f32, tag="rcp")
            nc.vector.reciprocal(rcp[:], den[:])
            nc.vector.tensor_tensor(vg[g][:, 1:2], asum[:], rcp[:], Alu.mult)
            nc.vector.tensor_scalar(
                vg[g][:, 2:3], c_g[:], 0.5, None, Alu.is_ge
            )
        # combine the 4 groups -> out[128, 3]
        v01 = accp.tile([P, 3], f32, tag="v01")
        v23 = accp.tile([P, 3], f32, tag="v23")
        nc.vector.tensor_tensor(v01[:], vg[0][:], vg[1][:], Alu.add)
        nc.vector.tensor_tensor(v23[:], vg[2][:], vg[3][:], Alu.add)
        nc.vector.tensor_tensor(v01[:], v01[:], v23[:], Alu.add)
        nc.sync.dma_start(out[:], v01[:])

    nc.compile()
    return nc


def get_nc():
    global _CACHED_NC
    if _CACHED_NC is None:
        _CACHED_NC = _build()
    return _CACHED_NC


def _make_in_maps(predictions, targets):
    p = np.asarray(predictions, dtype=np.float32).reshape(B, FREE)
    t = np.asarray(targets, dtype=np.float32).reshape(B, FREE)
    in_maps = []
    for i in range(N_CORES):
        sl = slice(i * B_CORE, (i + 1) * B_CORE)
        in_maps.append(
            {
                "predictions": np.ascontiguousarray(p[sl]),
                "targets": np.ascontiguousarray(t[sl]),
            }
        )
    return in_maps


def _combine(core_outs):
    vals = np.stack(core_outs).astype(np.float64)  # (8, 128, 3)
    csq_sum = vals[..., 0].sum()
    mse_sum = vals[..., 1].sum()
    n_valid = vals[..., 2].sum()
    count_loss = csq_sum / B
    atl = mse_sum / max(n_valid, 1.0)
    total = W_ACTION_COUNT * count_loss + (
        W_ACTION_TENSOR * atl if n_valid > 0 else 0.0
    )
    return np.array(total, dtype=np.float32)


def _ensure_ntff_hook():
    """The agent image's antenv package lacks axon_hooks, so the boot-time
    NTFF hook registration silently degrades. Recreate the module shim and
    register the ctypes hook so trace=True produces exec_time_ns."""
    import sys
    import types

    try:
        from antenv.axon_hooks import get_axon_ntff_profile_hook  # noqa: F401
        return
    except ImportError:
        pass
    mod = types.ModuleType("antenv.axon_hooks")
    _hook = [None]
    mod.set_axon_ntff_profile_hook = lambda h: _hook.__setitem__(0, h)
    mod.get_axon_ntff_profile_hook = lambda: _hook[0]
    sys.modules["antenv.axon_hooks"] = mod
    import antenv

    antenv.axon_hooks = mod
    try:
        from trn_agent_boot.trn_boot import _ntff_profile_via_ctypes

        mod.set_axon_ntff_profile_hook(
            _ntff_profile_via_ctypes("/opt/axon/libaxon_pjrt.so")
        )
    except Exception:
        pass


_STAGED_CACHE = {}


def _staged_exec(nc, in_maps, trace=False):
    """Execute the compiled Bass module via PJRT with inputs pre-staged onto
    the devices (device_put + block) before the NEFF launch.  The stock
    run_bass_via_pjrt path feeds numpy arrays straight into jit, so the
    537 MB of host->device traffic overlaps the NEFF execution; core 0
    (which receives its shard first and shares an HBM stack with core 1)
    then executes under HBM contention and runs ~20% slower.  Staging first
    keeps the measured kernel window clean."""
    import jax
    import numpy as np
    from jax.sharding import Mesh, NamedSharding, PartitionSpec
    from jax.experimental.shard_map import shard_map
    from concourse import bass2jax, mybir
    from concourse.bass2jax import _bass_exec_p, install_neuronx_cc_hook

    install_neuronx_cc_hook()
    n_cores = len(in_maps)

    partition_name = (
        nc.partition_id_tensor.name if nc.partition_id_tensor else None
    )
    in_names, out_names, out_avals, zero_outs = [], [], [], []
    for alloc in nc.m.functions[0].allocations:
        if not isinstance(alloc, mybir.MemoryLocationSet):
            continue
        name = alloc.memorylocations[0].name
        if alloc.kind == "ExternalInput":
            if name != partition_name:
                in_names.append(name)
        elif alloc.kind == "ExternalOutput":
            shape = tuple(alloc.tensor_shape)
            dtype = mybir.dt.np(alloc.dtype)
            out_names.append(name)
            out_avals.append(jax.core.ShapedArray(shape, dtype))
            zero_outs.append(np.zeros(shape, dtype))
    n_params = len(in_names)
    all_in_names = in_names + out_names
    if partition_name is not None:
        all_in_names.append(partition_name)
    donate = tuple(range(n_params, n_params + len(out_names)))

    def _body(*args):
        operands = list(args)
        if partition_name is not None:
            operands.append(bass2jax.partition_id_tensor())
        outs = _bass_exec_p.bind(
            *operands,
            out_avals=tuple(out_avals),
            in_names=tuple(all_in_names),
            out_names=tuple(out_names),
            lowering_input_output_aliases=(),
            sim_require_finite=True,
            sim_require_nnan=True,
            nc=nc,
        )
        return tuple(outs)

    devices = jax.devices()[:n_cores]
    mesh = Mesh(np.asarray(devices), ("core",))
    sharding = NamedSharding(mesh, PartitionSpec("core"))
    if id(nc) not in _STAGED_CACHE:
        sharded = jax.jit(
            shard_map(
                _body,
                mesh=mesh,
                in_specs=(PartitionSpec("core"),) * (n_params + len(out_names)),
                out_specs=(PartitionSpec("core"),) * len(out_names),
                check_rep=False,
            ),
            donate_argnums=donate,
            keep_unused=True,
        )
        _STAGED_CACHE[id(nc)] = sharded
    sharded = _STAGED_CACHE[id(nc)]

    concat_in = [
        np.concatenate([np.asarray(m[name]) for m in in_maps], axis=0)
        for name in in_names
    ]
    staged = [jax.device_put(a, sharding) for a in concat_in]
    staged_zeros = [
        jax.device_put(
            np.zeros((n_cores * z.shape[0], *z.shape[1:]), z.dtype), sharding
        )
        for z in zero_outs
    ]
    jax.block_until_ready(staged)
    jax.block_until_ready(staged_zeros)

    out_arrs = sharded(*staged, *staged_zeros)
    jax.block_until_ready(out_arrs)
    return [
        {
            name: np.asarray(out_arrs[i]).reshape(n_cores, *out_avals[i].shape)[c]
            for i, name in enumerate(out_names)
        }
        for c in range(n_cores)
    ]


def _run_staged(nc, in_maps, trace=False, trace_cores=None, warmup=False):
    """Staged execution; with trace=True, captures NTFF on a warm, quiet run."""
    import glob
    import tempfile

    from concourse import bass_utils

    if not trace:
        results = _staged_exec(nc, in_maps)
        return bass_utils.BassKernelResults(
            results=results,
            instructions_and_trace=None,
            profile_json=None,
            exec_time_ns=None,
        )

    _ensure_ntff_hook()
    from antenv.axon_hooks import get_axon_ntff_profile_hook

    hook = get_axon_ntff_profile_hook()
    if hook is None:
        raise RuntimeError("no ntff hook")
    if warmup:
        _staged_exec(nc, in_maps)  # compile + warm the devices
    neff_dir = tempfile.mkdtemp()
    idxs = list(trace_cores) if trace_cores is not None else list(range(N_CORES))
    with hook(neff_dir, idxs):
        results = _staged_exec(nc, in_maps)
    ntffs = glob.glob(os.path.join(neff_dir, "*_body*.ntff"))
    if not ntffs:
        return bass_utils.BassKernelResults(
            results=results,
            instructions_and_trace=None,
            profile_json=None,
            exec_time_ns=None,
        )
    import gauge.profiler
    from concourse._compat import FishPath

    sharepath = bass_utils.upload_artifacts(neff_dir)
    profile = gauge.profiler.Profile(
        profile_path=FishPath(neff_dir),
        kernel_dev_mode=True,
        profile_on_exit=False,
        bass_kernel=nc.m,
        offline_processing=True,
        fname="*_body*",
        metadata={"artifacts_path": sharepath},
    )
    perf = bass_utils._process_ntff_profile(
        profile,
        neff_dir,
        nc,
        list(range(N_CORES)),
        trace_cores,
        False,
        {},
        trace_events=False,
    )
    return perf.as_bass_kernel_results(results)


def run(predictions, targets, trace=False, staged=None, trace_cores=None,
        warmup=False, **spmd_kwargs):
    """Run on the 8 NeuronCores; returns (output, BassKernelResults)."""
    from concourse.bass_utils import run_bass_kernel_spmd

    nc = get_nc()
    in_maps = _make_in_maps(predictions, targets)

    trace = trace or os.environ.get("BASS_TRACE", "") == "1"
    if staged is None:
        staged = os.environ.get("ATL_STAGED", "1") == "1"
    if staged:
        try:
            res = _run_staged(
                nc, in_maps, trace=trace, trace_cores=trace_cores, warmup=warmup
            )
            out = _combine([r["out"] for r in res.results])
            return out, res
        except Exception:
            import traceback

            traceback.print_exc()

    if trace:
        _ensure_ntff_hook()
    res = run_bass_kernel_spmd(
        nc, in_maps, core_ids=list(range(N_CORES)), trace=trace, **spmd_kwargs
    )
    out = _combine([r["out"] for r in res.results])
    return out, res


def kernel(predictions, targets):
    out, _ = run(predictions, targets, trace=False)
    return out


if __name__ == "__main__":
    np.random.seed(0)
    preds = np.random.randn(B, ROWS, F).astype(np.float32)
    targs = np.random.randn(B, ROWS, F).astype(np.float32)
    counts = np.random.randint(0, A + 1, size=B).astype(np.float32)
    targs[:, 0, 0] = counts
    print(kernel(preds, targs))
